# revision 1
# baseline (speedup 1.0000x reference)
"""Compound loss (dice + focal + edge) kernel for Trainium2, 8-core data-parallel.

Shapes hardcoded: inputs [8, 11, 512, 512] f32, targets [8, 512, 512] int.
Each NeuronCore processes one batch sample and emits per-class partial sums
(stats [128, 268] f32); the tiny cross-batch combination happens on host.

Design notes:
- row-tiles of 128 rows; layout [h=partition, (c,w)=free].
- softmax probs P = exp(X) * (1/sum_c exp(X)) without max-subtraction (randn).
- onehot/pred one-hot handled as dense [128,11,512] f32 tiles.
- 3x3 convs on the class-mask bit-words (m = 1<<t): separable OR/AND built on
  GPSIMD, vertical shifts via a DRAM round-trip of the word planes.
- per-class sums via ScalarE (ACT) Copy+accum_out; focal via plane reduce.
- tensor_tensor_reduce is avoided entirely (crashes HW via this runtime).
"""

import sys

sys.path.insert(0, "/opt/trn_rl_repo")

import functools
import numpy as np

B, C, H, W = 8, 11, 512, 512
P = 128
NT = H // P
EPS = 1e-6
FOCAL_ALPHA = 0.25
E1 = float(np.exp(-1.0))
ES = float(np.exp(-np.sqrt(2.0)))

PER_TILE = 6 * C + 1
NCOL = NT * PER_TILE  # 268


def _col(k, q, c=0):
    base = k * PER_TILE
    if q == 6:
        return base + 6 * C
    return base + q * C + c


@functools.cache
def _build():
    import concourse.bacc as bacc
    from concourse import mybir, tile

    f32 = mybir.dt.float32
    i32 = mybir.dt.int32
    A = mybir.AluOpType
    AF = mybir.ActivationFunctionType
    AX = mybir.AxisListType.X

    nc = bacc.Bacc(None, target_bir_lowering=False)
    xin = nc.dram_tensor("inputs", [C, H, W], f32, kind="ExternalInput")
    tin = nc.dram_tensor("targets", [H, W], i32, kind="ExternalInput")
    out = nc.dram_tensor("stats", [P, NCOL], f32, kind="ExternalOutput")

    with tile.TileContext(nc) as tc:
        with (
            tc.tile_pool(name="const", bufs=1) as cpool,
            tc.tile_pool(name="dram", bufs=1, space="DRAM") as dpool,
            tc.tile_pool(name="big", bufs=1) as bpool,
            tc.tile_pool(name="xbuf", bufs=2) as xpool,
            tc.tile_pool(name="pl", bufs=1) as pp,
            tc.tile_pool(name="w2", bufs=2) as pp2,
        ):
            ones_i = cpool.tile([P, W], i32)
            nc.vector.memset(ones_i[:], 1)
            stats = cpool.tile([P, NCOL], f32)

            d_mw = dpool.tile([H, W], i32)
            d_rw = dpool.tile([H, W], i32)
            d_ro = dpool.tile([H, W], i32)
            d_ar = dpool.tile([H, W], i32)

            # ---------- phase 1: word planes -> DRAM
            Ts = []
            for k in range(NT):
                h0 = k * P
                T = cpool.tile([P, W], i32, tag=f"T{k}")
                nc.sync.dma_start(T[:], tin[h0 : h0 + P, :])
                Ts.append(T)

                mw = pp.tile([P, W], i32, tag="p1mw")
                nc.vector.tensor_tensor(mw[:], ones_i[:], T[:], A.logical_shift_left)
                rw = pp.tile([P, W], i32, tag="p1rw")
                nc.vector.memset(rw[:, 0:1], 0)
                nc.vector.tensor_copy(rw[:, 1:W], mw[:, 0 : W - 1])
                nc.vector.tensor_tensor(
                    rw[:, 0 : W - 1], rw[:, 0 : W - 1], mw[:, 1:W], A.bitwise_or
                )
                ro = pp.tile([P, W], i32, tag="p1ro")
                nc.vector.tensor_tensor(ro[:], rw[:], mw[:], A.bitwise_or)
                ar = pp.tile([P, W], i32, tag="p1ar")
                nc.vector.memset(ar[:, 0:1], 0)
                nc.vector.memset(ar[:, W - 1 : W], 0)
                nc.vector.tensor_tensor(
                    ar[:, 1 : W - 1], mw[:, 1 : W - 1], mw[:, 0 : W - 2], A.bitwise_and
                )
                nc.vector.tensor_tensor(
                    ar[:, 1 : W - 1], ar[:, 1 : W - 1], mw[:, 2:W], A.bitwise_and
                )
                nc.sync.dma_start(d_mw[h0 : h0 + P, :], mw[:])
                nc.sync.dma_start(d_rw[h0 : h0 + P, :], rw[:])
                nc.sync.dma_start(d_ro[h0 : h0 + P, :], ro[:])
                nc.sync.dma_start(d_ar[h0 : h0 + P, :], ar[:])

            # helper: load rows [h0+off .. h0+off+127] of a DRAM plane, zero-pad OOB
            def vload(dst, dplane, h0, off):
                lo = h0 + off
                if lo < 0:
                    nc.vector.memset(dst[:], 0)
                    nc.sync.dma_start(dst[1:P, :], dplane[0 : P - 1, :])
                elif lo + P > H:
                    nc.vector.memset(dst[:], 0)
                    nc.sync.dma_start(dst[0 : P - 1, :], dplane[lo : H, :])
                else:
                    nc.sync.dma_start(dst[:], dplane[lo : lo + P, :])

            # ---------- phase 2
            for k in range(NT):
                h0 = k * P
                T = Ts[k]

                Xt = xpool.tile([P, C, W], f32, tag="X")
                nc.sync.dma_start(
                    Xt[:], xin[:, h0 : h0 + P, :].rearrange("c h w -> h c w")
                )

                E = bpool.tile([P, C, W], f32, tag="E")
                nc.scalar.activation(E[:], Xt[:], AF.Exp)
                Dn = pp.tile([P, W], f32, tag="Dn")
                nc.vector.reduce_sum(Dn[:], E[:].transpose([0, 2, 1]), axis=AX)
                r = pp.tile([P, W], f32, tag="r")
                nc.vector.reciprocal(r[:], Dn[:])
                # P = E * r  (in place: E now holds probs)
                nc.vector.tensor_tensor(
                    E[:], E[:], r[:].unsqueeze(1).broadcast_to([P, C, W]), A.mult
                )
                Pr = E

                OH = bpool.tile([P, C, W], f32, tag="OH")
                for c in range(C):
                    nc.vector.tensor_scalar(OH[:, c, :], T[:], c, None, A.is_equal)

                # Q = OH * P (into X slot; X dead after exp)
                Q = Xt
                nc.vector.tensor_tensor(Q[:], OH[:], Pr[:], A.mult)
                pt = pp.tile([P, W], f32, tag="pt")
                nc.vector.reduce_sum(pt[:], Q[:].transpose([0, 2, 1]), axis=AX)

                Pmax = pp.tile([P, W], f32, tag="Pmax")
                nc.vector.reduce_max(Pmax[:], Pr[:].transpose([0, 2, 1]), axis=AX)
                PRED = bpool.tile([P, C, W], f32, tag="PRED")
                nc.vector.tensor_tensor(
                    PRED[:], Pr[:], Pmax[:].unsqueeze(1).broadcast_to([P, C, W]),
                    A.is_equal,
                )
                npe = pp.tile([P, W], f32, tag="npe")
                nc.vector.tensor_tensor(npe[:], pt[:], Pmax[:], A.is_equal)
                nc.vector.tensor_scalar(npe[:], npe[:], -1.0, 1.0, A.mult, A.add)

                # per-class sums: soh, inter, sumP (ScalarE accum)
                scrA = pp.tile([P, W], f32, tag="scrA")
                scrB = pp.tile([P, W], f32, tag="scrB")
                scrC = pp.tile([P, W], f32, tag="scrC")
                for c in range(C):
                    nc.scalar.activation(
                        scrA[:], OH[:, c, :], AF.Copy,
                        accum_out=stats[:, _col(k, 0, c) : _col(k, 0, c) + 1],
                    )
                    nc.scalar.activation(
                        scrB[:], Q[:, c, :], AF.Copy,
                        accum_out=stats[:, _col(k, 1, c) : _col(k, 1, c) + 1],
                    )
                    nc.scalar.activation(
                        scrC[:], Pr[:, c, :], AF.Copy,
                        accum_out=stats[:, _col(k, 2, c) : _col(k, 2, c) + 1],
                    )

                # PWQ = PRED * 2^c (in place, ScalarE)
                for c in range(C):
                    nc.scalar.activation(
                        PRED[:, c, :], PRED[:, c, :], AF.Copy, scale=float(1 << c)
                    )
                PWQ = PRED
                pmf = pp.tile([P, W], f32, tag="pmf")
                nc.vector.reduce_sum(pmf[:], PWQ[:].transpose([0, 2, 1]), axis=AX)
                pmi = pp.tile([P, W], i32, tag="pmi")
                nc.vector.tensor_copy(pmi[:], pmf[:])

                # focal
                nc.vector.tensor_scalar_max(pt[:], pt[:], 1e-7)
                Lp = pp.tile([P, W], f32, tag="Lp")
                nc.scalar.activation(Lp[:], pt[:], AF.Ln)
                u2 = pp.tile([P, W], f32, tag="u2")
                nc.scalar.activation(u2[:], pt[:], AF.Square, bias=1.0, scale=-1.0)
                nc.vector.tensor_tensor(u2[:], u2[:], Lp[:], A.mult)
                nc.vector.reduce_sum(
                    stats[:, _col(k, 6) : _col(k, 6) + 1], u2[:], axis=AX
                )

                # words: or8 / an9 / or4 from DRAM planes
                vu = pp2.tile([P, W], i32, tag="vu")
                vd = pp2.tile([P, W], i32, tag="vd")
                vc = pp2.tile([P, W], i32, tag="vc")
                or8 = pp.tile([P, W], i32, tag="or8")
                vload(vc, d_ro, h0, 0)
                vload(vu, d_ro, h0, -1)
                vload(vd, d_ro, h0, 1)
                nc.vector.tensor_tensor(or8[:], vc[:], vu[:], A.bitwise_or)
                nc.vector.tensor_tensor(or8[:], or8[:], vd[:], A.bitwise_or)

                an9 = pp.tile([P, W], i32, tag="an9")
                vu2 = pp2.tile([P, W], i32, tag="vu")
                vd2 = pp2.tile([P, W], i32, tag="vd")
                vc2 = pp2.tile([P, W], i32, tag="vc")
                vload(vc2, d_ar, h0, 0)
                vload(vu2, d_ar, h0, -1)
                vload(vd2, d_ar, h0, 1)
                nc.vector.tensor_tensor(an9[:], vc2[:], vu2[:], A.bitwise_and)
                nc.vector.tensor_tensor(an9[:], an9[:], vd2[:], A.bitwise_and)

                or4 = pp.tile([P, W], i32, tag="or4")
                mwc = pp.tile([P, W], i32, tag="mwc")
                vu3 = pp2.tile([P, W], i32, tag="vu")
                vd3 = pp2.tile([P, W], i32, tag="vd")
                vload(mwc, d_mw, h0, 0)
                vload(vu3, d_mw, h0, -1)
                vload(vd3, d_mw, h0, 1)
                vc3 = pp2.tile([P, W], i32, tag="vc")
                vload(vc3, d_rw, h0, 0)
                nc.vector.tensor_tensor(or4[:], vc3[:], vu3[:], A.bitwise_or)
                nc.vector.tensor_tensor(or4[:], or4[:], vd3[:], A.bitwise_or)

                # b9t before an9 is overwritten by BW
                b9t = pp.tile([P, W], f32, tag="b9t")
                nc.vector.tensor_tensor(b9t[:], an9[:], mwc[:], A.is_equal)
                # BW = ~an9 & or8  (in place into an9)
                nc.vector.tensor_scalar(an9[:], an9[:], -1, None, A.bitwise_xor)
                nc.vector.tensor_tensor(an9[:], an9[:], or8[:], A.bitwise_and)
                BW = an9

                # gA plane = npe * (1 - b9t)
                nc.vector.tensor_scalar(b9t[:], b9t[:], -1.0, 1.0, A.mult, A.add)
                gAp = pp.tile([P, W], f32, tag="gAp")
                nc.vector.tensor_tensor(gAp[:], npe[:], b9t[:], A.mult)

                # B0p / O4p bit-gathers
                ti = pp.tile([P, W], i32, tag="ti")
                nc.vector.tensor_tensor(ti[:], or8[:], pmi[:], A.bitwise_and)
                B0p = pp.tile([P, W], f32, tag="B0p")
                nc.vector.tensor_scalar(B0p[:], ti[:], 0, None, A.is_gt)
                ti2 = pp.tile([P, W], i32, tag="ti2")
                nc.vector.tensor_tensor(ti2[:], or4[:], pmi[:], A.bitwise_and)
                O4p = pp.tile([P, W], f32, tag="O4p")
                nc.vector.tensor_scalar(O4p[:], ti2[:], 0, None, A.is_gt)

                # g23 = (ES + (E1-ES)*O4p) * B0p * npe
                g23 = pp.tile([P, W], f32, tag="g23")
                nc.vector.tensor_scalar(g23[:], O4p[:], E1 - ES, ES, A.mult, A.add)
                nc.vector.tensor_tensor(g23[:], g23[:], B0p[:], A.mult)
                nc.vector.tensor_tensor(g23[:], g23[:], npe[:], A.mult)

                # GR = OH * gA (in place), NR = PWQ * g23 (in place) on GPSIMD
                nc.gpsimd.tensor_tensor(
                    OH[:], OH[:], gAp[:].unsqueeze(1).broadcast_to([P, C, W]), A.mult
                )
                nc.gpsimd.tensor_tensor(
                    PWQ[:], PWQ[:], g23[:].unsqueeze(1).broadcast_to([P, C, W]), A.mult
                )
                scrD = pp.tile([P, W], f32, tag="scrD")
                scrE = pp.tile([P, W], f32, tag="scrE")
                for c in range(C):
                    nc.scalar.activation(
                        scrD[:], OH[:, c, :], AF.Copy,
                        accum_out=stats[:, _col(k, 3, c) : _col(k, 3, c) + 1],
                    )
                    nc.scalar.activation(
                        scrE[:], PWQ[:, c, :], AF.Copy,
                        accum_out=stats[:, _col(k, 4, c) : _col(k, 4, c) + 1],
                    )

                # ne: (BW & 2^c) summed (ScalarE accum, int->f32)
                for c in range(C):
                    scri = pp2.tile([P, W], i32, tag="scri")
                    scrF = pp2.tile([P, W], f32, tag="scrF")
                    nc.vector.tensor_scalar(
                        scri[:], BW[:], 1 << c, None, A.bitwise_and
                    )
                    nc.scalar.activation(
                        scrF[:], scri[:], AF.Copy,
                        accum_out=stats[:, _col(k, 5, c) : _col(k, 5, c) + 1],
                    )

            nc.sync.dma_start(out[:], stats[:])

    nc.compile()
    return nc


def _host_combine(stats_list):
    soh = np.zeros((B, C)); inter = np.zeros((B, C)); sumP = np.zeros((B, C))
    gA = np.zeros((B, C)); n23 = np.zeros((B, C)); ne = np.zeros((B, C))
    fsum = np.zeros(B)
    pw2 = 2.0 ** np.arange(C)
    for b in range(B):
        st = stats_list[b].astype(np.float64).sum(axis=0)
        for k in range(NT):
            soh[b] += st[_col(k, 0) : _col(k, 0) + C]
            inter[b] += st[_col(k, 1) : _col(k, 1) + C]
            sumP[b] += st[_col(k, 2) : _col(k, 2) + C]
            gA[b] += st[_col(k, 3) : _col(k, 3) + C]
            n23[b] += st[_col(k, 4) : _col(k, 4) + C] / pw2
            ne[b] += st[_col(k, 5) : _col(k, 5) + C] / pw2
            fsum[b] += st[_col(k, 6)]

    dice = (2.0 * inter + EPS) / (sumP + soh + EPS)
    cls = np.arange(C)
    cls_valid = (soh.sum(axis=0) > 0) & (cls != 0)
    nvalid = int(cls_valid.sum())
    dice_score = (dice.mean(axis=0) * cls_valid).sum() / max(nvalid, 1)
    dice_loss = (1.0 - dice_score) if nvalid > 0 else 0.0

    focal_loss = -FOCAL_ALPHA * fsum.sum() / (B * H * W)

    werr = gA + n23
    class_loss = werr / np.maximum(ne, 1.0)
    valid_bc = (soh > 0) & (cls[None, :] != 0)
    nvalid_b = valid_bc.sum(axis=1)
    sample = (class_loss * valid_bc).sum(axis=1) / np.maximum(nvalid_b, 1)
    edge_loss = float(np.where(nvalid_b > 0, sample, 0.0).mean())

    total = dice_loss + focal_loss + edge_loss
    return (
        np.float32(total),
        np.float32(dice_loss),
        np.float32(focal_loss),
        np.float32(edge_loss),
    )


def kernel(inputs, targets):
    from concourse.bass_utils import run_bass_kernel_spmd

    inputs = np.ascontiguousarray(np.asarray(inputs, dtype=np.float32))
    tgt = np.ascontiguousarray(np.asarray(targets).astype(np.int32))

    nc = _build()
    in_maps = [{"inputs": inputs[b], "targets": tgt[b]} for b in range(B)]
    res = run_bass_kernel_spmd(nc, in_maps, core_ids=list(range(B)))
    return _host_combine([res.results[b]["stats"] for b in range(B)])



# revision 10
# speedup vs baseline: 1.5989x; 1.5989x over previous
"""Compound loss (dice + focal + edge) kernel for Trainium2, 8-core data-parallel.

Shapes hardcoded: inputs [8, 11, 512, 512] f32, targets [8, 512, 512] int.
Each NeuronCore processes one batch sample; per-class reductions run on the
TensorEngine (one-hot-column stationary matmuls accumulating into PSUM
[11, 512] banks); the tiny cross-batch combination happens on host.

v2 design (vs v1 at 464 us: Vector 89% / Scalar 72% busy):
- all per-class column sums -> TensorE matmul (lhsT = [128,11] one-hot col c,
  rhs = quantity plane [128,512], PSUM accumulates across row-tiles).
- strided C-reductions -> contiguous pairwise trees on bf16 (2x DVE rate).
- argmax (max tree + is_equal) on GpSimd in f32 (exact, engine otherwise idle).
- pred bit-word via ScalarE per-class scale 2^c into i16 + Vector tree (exact).
- 3x3 word-plane convs: horizontal ops on i16 words; vertical neighbors by
  re-loading row-shifted target slices from DRAM (no round-trip of planes).
- host passes targets pre-cast as i16 and bf16 to skip on-device casts.
"""

import sys

sys.path.insert(0, "/opt/trn_rl_repo")

import functools
import numpy as np

B, C, H, W = 8, 11, 512, 512
P = 128
NT = H // P
EPS = 1e-6
FOCAL_ALPHA = 0.25
E1 = float(np.exp(-1.0))
ES = float(np.exp(-np.sqrt(2.0)))

NQ = 6  # soh, inter, sumP, gA, NR, ne


@functools.cache
def _build():
    import concourse.bacc as bacc
    from concourse import mybir, tile

    f32 = mybir.dt.float32
    bf16 = mybir.dt.bfloat16
    i16 = mybir.dt.int16
    A = mybir.AluOpType
    AF = mybir.ActivationFunctionType
    AX = mybir.AxisListType.X

    nc = bacc.Bacc(None, target_bir_lowering=False)
    xin = nc.dram_tensor("inputs", [C, H, W], f32, kind="ExternalInput")
    t16 = nc.dram_tensor("t16", [H, W], i16, kind="ExternalInput")
    tbf = nc.dram_tensor("tbf", [H, W], bf16, kind="ExternalInput")
    pso = nc.dram_tensor("psums", [C, NQ * W], f32, kind="ExternalOutput")
    sto = nc.dram_tensor("stats", [P, NT], f32, kind="ExternalOutput")

    with tile.TileContext(nc) as tc:
        with (
            tc.tile_pool(name="const", bufs=1) as cpool,
            tc.tile_pool(name="xbuf", bufs=2) as xpool,
            tc.tile_pool(name="ebuf", bufs=2) as epool,
            tc.tile_pool(name="obuf", bufs=1) as opool,
            tc.tile_pool(name="qbuf", bufs=1) as qpool,
            tc.tile_pool(name="pbuf", bufs=1) as ppool,
            tc.tile_pool(name="wbuf", bufs=1) as wpool,
            tc.tile_pool(name="bbuf", bufs=1) as bbuf,
            tc.tile_pool(name="pl", bufs=2) as pp,
            tc.tile_pool(name="tb", bufs=1) as tpool,
            tc.psum_pool(name="acc", bufs=1) as psp,
        ):
            ones_i = cpool.tile([P, W], i16)
            nc.vector.memset(ones_i[:], 1)
            # IDE[:, c, :] = one-hot row pattern: column c ones (stationary)
            IDE = cpool.tile([P, C, C], bf16)
            nc.vector.memset(IDE[:], 0.0)
            for c in range(C):
                nc.vector.memset(IDE[:, c, c : c + 1], 1.0)
            stats = cpool.tile([P, NT], f32)

            ps = [
                psp.tile([C, W], f32, tag=f"ps{q}", name=f"ps{q}")
                for q in range(NQ)
            ]

            def mm(q, lhs_c, rhs, k, c):
                nc.tensor.matmul(
                    ps[q][:],
                    IDE[:, lhs_c, :],
                    rhs,
                    start=(k == 0 and c == 0),
                    stop=(k == NT - 1 and c == C - 1),
                )

            for k in range(NT):
                h0 = k * P

                Xt = xpool.tile([P, C, W], f32, tag="X")
                nc.sync.dma_start(
                    Xt[:], xin[:, h0 : h0 + P, :].rearrange("c h w -> h c w")
                )
                T = tpool.tile([P, W], i16, tag="T")
                nc.sync.dma_start(T[:], t16[h0 : h0 + P, :])
                Tb = tpool.tile([P, W], bf16, tag="Tb")
                nc.sync.dma_start(Tb[:], tbf[h0 : h0 + P, :])
                TUD = tpool.tile([P, 2, W], i16, tag="TUD")
                if k == 0:
                    nc.vector.memset(TUD[0:1, 0, :], 0)
                    nc.sync.dma_start(TUD[1:P, 0, :], t16[0 : P - 1, :])
                else:
                    nc.sync.dma_start(TUD[:, 0, :], t16[h0 - 1 : h0 + P - 1, :])
                if k == NT - 1:
                    nc.vector.memset(TUD[:, 1, :], 0)
                    nc.sync.dma_start(TUD[0 : P - 1, 1, :], t16[h0 + 1 : H, :])
                else:
                    nc.sync.dma_start(TUD[:, 1, :], t16[h0 + 1 : h0 + P + 1, :])

                # ---- softmax pieces ----
                E = epool.tile([P, C, W], bf16, tag="E")
                nc.scalar.activation(E[:], Xt[:], AF.Exp)
                s5 = pp.tile([P, 5, W], bf16, tag="s5", bufs=1)
                nc.vector.tensor_tensor(s5[:], E[:, 0:5, :], E[:, 5:10, :], A.add)
                s2 = pp.tile([P, 2, W], bf16, tag="s2", bufs=1)
                nc.vector.tensor_tensor(s2[:], s5[:, 0:2, :], s5[:, 2:4, :], A.add)
                Dn = pp.tile([P, W], bf16, tag="Dn")
                nc.vector.tensor_tensor(Dn[:], s2[:, 0, :], s2[:, 1, :], A.add)
                nc.vector.tensor_tensor(Dn[:], Dn[:], s5[:, 4, :], A.add)
                nc.vector.tensor_tensor(Dn[:], Dn[:], E[:, 10, :], A.add)
                lnD = pp.tile([P, W], bf16, tag="lnD")
                nc.scalar.activation(lnD[:], Dn[:], AF.Ln)
                r = pp.tile([P, W], bf16, tag="r")
                nc.scalar.activation(r[:], lnD[:], AF.Exp, scale=-1.0)
                # Pr = E * r (in place)
                nc.vector.tensor_tensor(
                    E[:], E[:], r[:].unsqueeze(1).broadcast_to([P, C, W]), A.mult
                )
                Pr = E

                OH = opool.tile([P, C, W], bf16, tag="OH")
                for c in range(C):
                    nc.vector.tensor_scalar(
                        OH[:, c, :], Tb[:], float(c), None, A.is_equal
                    )
                Q = qpool.tile([P, C, W], bf16, tag="Q")
                nc.vector.tensor_tensor(Q[:], OH[:], Pr[:], A.mult)

                # per-class sums that don't depend on later products
                for c in range(C):
                    mm(0, c, OH[:, c, :], k, c)
                for c in range(C):
                    mm(1, c, Q[:, c, :], k, c)
                for c in range(C):
                    mm(2, c, Pr[:, c, :], k, c)

                # pt = sum_c Q (exact: one nonzero per pixel)
                p5 = pp.tile([P, 5, W], bf16, tag="p5", bufs=1)
                nc.vector.tensor_tensor(p5[:], Q[:, 0:5, :], Q[:, 5:10, :], A.add)
                p2 = pp.tile([P, 2, W], bf16, tag="p2", bufs=1)
                nc.vector.tensor_tensor(p2[:], p5[:, 0:2, :], p5[:, 2:4, :], A.add)
                pt = pp.tile([P, W], bf16, tag="pt")
                nc.vector.tensor_tensor(pt[:], p2[:, 0, :], p2[:, 1, :], A.add)
                nc.vector.tensor_tensor(pt[:], pt[:], p5[:, 4, :], A.add)
                nc.vector.tensor_tensor(pt[:], pt[:], Q[:, 10, :], A.add)
                nc.vector.tensor_scalar_max(pt[:], pt[:], 1e-7)
                Lp = pp.tile([P, W], bf16, tag="Lp")
                nc.scalar.activation(Lp[:], pt[:], AF.Ln)
                u2 = pp.tile([P, W], bf16, tag="u2")
                nc.scalar.activation(u2[:], pt[:], AF.Square, bias=1.0, scale=-1.0)
                fpl = pp.tile([P, W], bf16, tag="fpl", bufs=1)
                nc.vector.tensor_tensor(fpl[:], u2[:], Lp[:], A.mult)
                nc.vector.reduce_sum(stats[:, k : k + 1], fpl[:], axis=AX)

                # ---- argmax via bf16 max tree over Pr (ties: multi-hot,
                # verified 6e-4 end-to-end error in numpy) ----
                m5 = pp.tile([P, 5, W], bf16, tag="m5", bufs=1)
                nc.vector.tensor_tensor(m5[:], Pr[:, 0:5, :], Pr[:, 5:10, :], A.max)
                nc.vector.tensor_tensor(
                    m5[:, 0:2, :], m5[:, 0:2, :], m5[:, 2:4, :], A.max
                )
                Em = pp.tile([P, W], bf16, tag="Em", bufs=1)
                nc.vector.tensor_tensor(Em[:], m5[:, 0, :], m5[:, 1, :], A.max)
                nc.vector.tensor_tensor(Em[:], Em[:], m5[:, 4, :], A.max)
                nc.vector.tensor_tensor(Em[:], Em[:], Pr[:, 10, :], A.max)
                PRED = ppool.tile([P, C, W], bf16, tag="PRED")
                nc.vector.tensor_tensor(
                    PRED[:], Pr[:], Em[:].unsqueeze(1).broadcast_to([P, C, W]),
                    A.is_equal,
                )
                # PWQ = PRED * 2^c as exact i16 words
                PWQ = wpool.tile([P, C, W], i16, tag="PWQ")
                for c in range(C):
                    nc.scalar.activation(
                        PWQ[:, c, :], PRED[:, c, :], AF.Copy, scale=float(1 << c)
                    )

                # ---- word planes (i16) ----
                mwUD = pp.tile([P, 2, W], i16, tag="mwUD", bufs=1)
                nc.vector.tensor_tensor(
                    mwUD[:], ones_i[:].unsqueeze(1).broadcast_to([P, 2, W]),
                    TUD[:], A.logical_shift_left,
                )
                roUD = pp.tile([P, 2, W], i16, tag="roUD", bufs=1)
                nc.vector.memset(roUD[:, :, 0:1], 0)
                nc.vector.tensor_copy(roUD[:, :, 1:W], mwUD[:, :, 0 : W - 1])
                nc.vector.tensor_tensor(roUD[:], roUD[:], mwUD[:], A.bitwise_or)
                nc.vector.tensor_tensor(
                    roUD[:, :, 0 : W - 1], roUD[:, :, 0 : W - 1],
                    mwUD[:, :, 1:W], A.bitwise_or,
                )
                arUD = pp.tile([P, 2, W], i16, tag="arUD", bufs=1)
                nc.vector.memset(arUD[:, :, 0:1], 0)
                nc.vector.memset(arUD[:, :, W - 1 : W], 0)
                nc.vector.tensor_tensor(
                    arUD[:, :, 1 : W - 1], mwUD[:, :, 1 : W - 1],
                    mwUD[:, :, 0 : W - 2], A.bitwise_and,
                )
                nc.vector.tensor_tensor(
                    arUD[:, :, 1 : W - 1], arUD[:, :, 1 : W - 1],
                    mwUD[:, :, 2:W], A.bitwise_and,
                )
                mwC = pp.tile([P, W], i16, tag="mwC", bufs=1)
                nc.vector.tensor_tensor(mwC[:], ones_i[:], T[:], A.logical_shift_left)
                rwC = pp.tile([P, W], i16, tag="rwC", bufs=1)
                nc.vector.memset(rwC[:, 0:1], 0)
                nc.vector.tensor_copy(rwC[:, 1:W], mwC[:, 0 : W - 1])
                nc.vector.tensor_tensor(
                    rwC[:, 0 : W - 1], rwC[:, 0 : W - 1], mwC[:, 1:W], A.bitwise_or
                )
                roC = pp.tile([P, W], i16, tag="roC", bufs=1)
                nc.vector.tensor_tensor(roC[:], rwC[:], mwC[:], A.bitwise_or)
                arC = pp.tile([P, W], i16, tag="arC", bufs=1)
                nc.vector.memset(arC[:, 0:1], 0)
                nc.vector.memset(arC[:, W - 1 : W], 0)
                nc.vector.tensor_tensor(
                    arC[:, 1 : W - 1], mwC[:, 1 : W - 1], mwC[:, 0 : W - 2],
                    A.bitwise_and,
                )
                nc.vector.tensor_tensor(
                    arC[:, 1 : W - 1], arC[:, 1 : W - 1], mwC[:, 2:W], A.bitwise_and
                )
                or8 = pp.tile([P, W], i16, tag="or8", bufs=1)
                nc.vector.tensor_tensor(or8[:], roUD[:, 0, :], roUD[:, 1, :], A.bitwise_or)
                nc.vector.tensor_tensor(or8[:], or8[:], roC[:], A.bitwise_or)
                an9 = pp.tile([P, W], i16, tag="an9", bufs=1)
                nc.vector.tensor_tensor(an9[:], arUD[:, 0, :], arUD[:, 1, :], A.bitwise_and)
                nc.vector.tensor_tensor(an9[:], an9[:], arC[:], A.bitwise_and)
                or4 = pp.tile([P, W], i16, tag="or4", bufs=1)
                nc.vector.tensor_tensor(or4[:], mwUD[:, 0, :], mwUD[:, 1, :], A.bitwise_or)
                nc.vector.tensor_tensor(or4[:], or4[:], rwC[:], A.bitwise_or)

                # pmi = 1 << pred (i16 tree over PWQ)
                w5 = pp.tile([P, 5, W], i16, tag="w5", bufs=1)
                nc.vector.tensor_tensor(w5[:], PWQ[:, 0:5, :], PWQ[:, 5:10, :], A.add)
                w2 = pp.tile([P, 2, W], i16, tag="w2", bufs=1)
                nc.vector.tensor_tensor(w2[:], w5[:, 0:2, :], w5[:, 2:4, :], A.add)
                pmi = pp.tile([P, W], i16, tag="pmi", bufs=1)
                nc.vector.tensor_tensor(pmi[:], w2[:, 0, :], w2[:, 1, :], A.add)
                nc.vector.tensor_tensor(pmi[:], pmi[:], w5[:, 4, :], A.add)
                nc.vector.tensor_tensor(pmi[:], pmi[:], PWQ[:, 10, :], A.add)

                # npe = [pred != t], gAp = npe * boundary_t
                ti = pp.tile([P, W], i16, tag="ti", bufs=1)
                nc.vector.tensor_tensor(ti[:], mwC[:], pmi[:], A.bitwise_and)
                npe = pp.tile([P, W], bf16, tag="npe", bufs=1)
                nc.vector.tensor_scalar(npe[:], ti[:], 0, None, A.is_equal)
                b9tp = pp.tile([P, W], bf16, tag="b9tp", bufs=1)
                nc.vector.tensor_tensor(b9tp[:], an9[:], mwC[:], A.not_equal)
                gAp = pp.tile([P, W], bf16, tag="gAp", bufs=1)
                nc.vector.tensor_tensor(gAp[:], npe[:], b9tp[:], A.mult)

                # g23 = (ES + (E1-ES)*O4) * B0 * npe
                o4a = pp.tile([P, W], i16, tag="o4a", bufs=1)
                nc.vector.tensor_tensor(o4a[:], or4[:], pmi[:], A.bitwise_and)
                g23 = pp.tile([P, W], bf16, tag="g23", bufs=1)
                nc.vector.tensor_scalar(g23[:], o4a[:], 0, None, A.is_gt)
                nc.vector.tensor_scalar(g23[:], g23[:], E1 - ES, ES, A.mult, A.add)
                b0a = pp.tile([P, W], i16, tag="b0a", bufs=1)
                nc.vector.tensor_tensor(b0a[:], or8[:], pmi[:], A.bitwise_and)
                b0v = pp.tile([P, W], bf16, tag="b0v", bufs=1)
                nc.vector.tensor_scalar(b0v[:], b0a[:], 0, None, A.is_gt)
                nc.vector.tensor_tensor(g23[:], g23[:], b0v[:], A.mult)
                nc.vector.tensor_tensor(g23[:], g23[:], npe[:], A.mult)

                # GR = OH * gAp (in place), NRt = PRED * g23 (in place)
                nc.gpsimd.tensor_tensor(
                    OH[:], OH[:], gAp[:].unsqueeze(1).broadcast_to([P, C, W]), A.mult
                )
                nc.gpsimd.tensor_tensor(
                    PRED[:], PRED[:], g23[:].unsqueeze(1).broadcast_to([P, C, W]),
                    A.mult,
                )
                for c in range(C):
                    mm(3, c, OH[:, c, :], k, c)
                for c in range(C):
                    mm(4, c, PRED[:, c, :], k, c)

                # ne: BW = ~an9 & or8, bit-sliced per class
                nn = pp.tile([P, W], i16, tag="nn", bufs=1)
                nc.vector.tensor_scalar(nn[:], an9[:], -1, None, A.bitwise_xor)
                BW = pp.tile([P, W], i16, tag="BW", bufs=1)
                nc.vector.tensor_tensor(BW[:], nn[:], or8[:], A.bitwise_and)
                BWbi = wpool.tile([P, C, W], i16, tag="PWQ", name="BWbi")
                for c in range(C):
                    nc.vector.tensor_scalar(
                        BWbi[:, c, :], BW[:], c, 1,
                        A.logical_shift_right, A.bitwise_and,
                    )
                BWb = bbuf.tile([P, C, W], bf16, tag="BWb")
                nc.vector.tensor_copy(BWb[:], BWbi[:])
                for c in range(C):
                    mm(5, c, BWb[:, c, :], k, c)

            # evacuate PSUM accumulators (chunked to keep SBUF small)
            for q in range(NQ):
                ev = pp.tile([C, W], f32, tag="ev", name="ev")
                nc.scalar.copy(ev[:], ps[q][:])
                nc.sync.dma_start(pso[:, q * W : (q + 1) * W], ev[:])
            nc.sync.dma_start(sto[:], stats[:])

    nc.compile()
    return nc


def _host_combine(results):
    soh = np.zeros((B, C)); inter = np.zeros((B, C)); sumP = np.zeros((B, C))
    gA = np.zeros((B, C)); NR = np.zeros((B, C)); ne = np.zeros((B, C))
    fsum = np.zeros(B)
    for b in range(B):
        psums = results[b]["psums"].astype(np.float64)  # [C, 6*W]
        soh[b] = psums[:, 0 * W : 1 * W].sum(axis=1)
        inter[b] = psums[:, 1 * W : 2 * W].sum(axis=1)
        sumP[b] = psums[:, 2 * W : 3 * W].sum(axis=1)
        gA[b] = psums[:, 3 * W : 4 * W].sum(axis=1)
        NR[b] = psums[:, 4 * W : 5 * W].sum(axis=1)
        ne[b] = psums[:, 5 * W : 6 * W].sum(axis=1)
        fsum[b] = results[b]["stats"].astype(np.float64).sum()

    cls = np.arange(C)
    dice = (2.0 * inter + EPS) / (sumP + soh + EPS)
    cls_valid = (soh.sum(axis=0) > 0) & (cls != 0)
    nvalid = int(cls_valid.sum())
    dice_score = (dice.mean(axis=0) * cls_valid).sum() / max(nvalid, 1)
    dice_loss = (1.0 - dice_score) if nvalid > 0 else 0.0

    focal_loss = -FOCAL_ALPHA * fsum.sum() / (B * H * W)

    werr = gA + NR
    class_loss = werr / np.maximum(ne, 1.0)
    valid_bc = (soh > 0) & (cls[None, :] != 0)
    nvalid_b = valid_bc.sum(axis=1)
    sample = (class_loss * valid_bc).sum(axis=1) / np.maximum(nvalid_b, 1)
    edge_loss = float(np.where(nvalid_b > 0, sample, 0.0).mean())

    total = dice_loss + focal_loss + edge_loss
    return (
        np.float32(total),
        np.float32(dice_loss),
        np.float32(focal_loss),
        np.float32(edge_loss),
    )


def kernel(inputs, targets):
    import ml_dtypes
    from concourse.bass_utils import run_bass_kernel_spmd

    inputs = np.ascontiguousarray(np.asarray(inputs, dtype=np.float32))
    tgt = np.asarray(targets)
    t16 = np.ascontiguousarray(tgt.astype(np.int16))
    tbf = np.ascontiguousarray(tgt.astype(ml_dtypes.bfloat16))

    nc = _build()
    in_maps = [
        {"inputs": inputs[b], "t16": t16[b], "tbf": tbf[b]} for b in range(B)
    ]
    res = run_bass_kernel_spmd(nc, in_maps, core_ids=list(range(B)))
    return _host_combine(res.results)


# revision 11
# speedup vs baseline: 1.6406x; 1.0261x over previous
"""Compound loss (dice + focal + edge) kernel for Trainium2, 8-core data-parallel.

Shapes hardcoded: inputs [8, 11, 512, 512] f32, targets [8, 512, 512] int.
Each NeuronCore processes one batch sample; per-class reductions run on the
TensorEngine (one-hot-column stationary matmuls accumulating into PSUM
[11, 512] banks); the tiny cross-batch combination happens on host.

v2 design (vs v1 at 464 us: Vector 89% / Scalar 72% busy):
- all per-class column sums -> TensorE matmul (lhsT = [128,11] one-hot col c,
  rhs = quantity plane [128,512], PSUM accumulates across row-tiles).
- strided C-reductions -> contiguous pairwise trees on bf16 (2x DVE rate).
- argmax (max tree + is_equal) on GpSimd in f32 (exact, engine otherwise idle).
- pred bit-word via ScalarE per-class scale 2^c into i16 + Vector tree (exact).
- 3x3 word-plane convs: horizontal ops on i16 words; vertical neighbors by
  re-loading row-shifted target slices from DRAM (no round-trip of planes).
- host passes targets pre-cast as i16 and bf16 to skip on-device casts.
"""

import sys

sys.path.insert(0, "/opt/trn_rl_repo")

import functools
import numpy as np

B, C, H, W = 8, 11, 512, 512
P = 128
NT = H // P
EPS = 1e-6
FOCAL_ALPHA = 0.25
E1 = float(np.exp(-1.0))
ES = float(np.exp(-np.sqrt(2.0)))

NQ = 6  # soh, inter, sumP, gA, NR, ne


@functools.cache
def _build():
    import concourse.bacc as bacc
    from concourse import mybir, tile

    f32 = mybir.dt.float32
    bf16 = mybir.dt.bfloat16
    i16 = mybir.dt.int16
    A = mybir.AluOpType
    AF = mybir.ActivationFunctionType
    AX = mybir.AxisListType.X

    nc = bacc.Bacc(None, target_bir_lowering=False)
    xin = nc.dram_tensor("inputs", [C, H, W], f32, kind="ExternalInput")
    t16 = nc.dram_tensor("t16", [H, W], i16, kind="ExternalInput")
    tbf = nc.dram_tensor("tbf", [H, W], bf16, kind="ExternalInput")
    pso = nc.dram_tensor("psums", [C, NQ * W], f32, kind="ExternalOutput")
    sto = nc.dram_tensor("stats", [P, NT], f32, kind="ExternalOutput")

    with tile.TileContext(nc) as tc:
        with (
            tc.tile_pool(name="const", bufs=1) as cpool,
            tc.tile_pool(name="xbuf", bufs=2) as xpool,
            tc.tile_pool(name="ebuf", bufs=2) as epool,
            tc.tile_pool(name="obuf", bufs=1) as opool,
            tc.tile_pool(name="qbuf", bufs=1) as qpool,
            tc.tile_pool(name="pbuf", bufs=1) as ppool,
            tc.tile_pool(name="wbuf", bufs=1) as wpool,
            tc.tile_pool(name="bbuf", bufs=1) as bbuf,
            tc.tile_pool(name="pl", bufs=2) as pp,
            tc.tile_pool(name="tb", bufs=1) as tpool,
            tc.psum_pool(name="acc", bufs=1) as psp,
        ):
            ones_i = cpool.tile([P, W], i16)
            nc.vector.memset(ones_i[:], 1)
            # IDE[:, c, :] = one-hot row pattern: column c ones (stationary)
            IDE = cpool.tile([P, C, C], bf16)
            nc.vector.memset(IDE[:], 0.0)
            for c in range(C):
                nc.vector.memset(IDE[:, c, c : c + 1], 1.0)
            stats = cpool.tile([P, NT], f32)

            ps = [
                psp.tile([C, W], f32, tag=f"ps{q}", name=f"ps{q}")
                for q in range(NQ)
            ]

            def mm(q, lhs_c, rhs, k, c):
                nc.tensor.matmul(
                    ps[q][:],
                    IDE[:, lhs_c, :],
                    rhs,
                    start=(k == 0 and c == 0),
                    stop=(k == NT - 1 and c == C - 1),
                )

            for k in range(NT):
                h0 = k * P

                Xt = xpool.tile([P, C, W], f32, tag="X")
                nc.sync.dma_start(
                    Xt[:], xin[:, h0 : h0 + P, :].rearrange("c h w -> h c w")
                )
                T = tpool.tile([P, W], i16, tag="T")
                nc.sync.dma_start(T[:], t16[h0 : h0 + P, :])
                Tb = tpool.tile([P, W], bf16, tag="Tb")
                nc.sync.dma_start(Tb[:], tbf[h0 : h0 + P, :])
                TUD = tpool.tile([P, 2, W], i16, tag="TUD")
                if k == 0:
                    nc.vector.memset(TUD[0:1, 0, :], 0)
                    nc.sync.dma_start(TUD[1:P, 0, :], t16[0 : P - 1, :])
                else:
                    nc.sync.dma_start(TUD[:, 0, :], t16[h0 - 1 : h0 + P - 1, :])
                if k == NT - 1:
                    nc.vector.memset(TUD[:, 1, :], 0)
                    nc.sync.dma_start(TUD[0 : P - 1, 1, :], t16[h0 + 1 : H, :])
                else:
                    nc.sync.dma_start(TUD[:, 1, :], t16[h0 + 1 : h0 + P + 1, :])

                # ---- softmax pieces ----
                E = epool.tile([P, C, W], bf16, tag="E")
                nc.scalar.activation(E[:], Xt[:], AF.Exp)
                s5 = pp.tile([P, 5, W], bf16, tag="s5", bufs=1)
                nc.vector.tensor_tensor(s5[:], E[:, 0:5, :], E[:, 5:10, :], A.add)
                s2 = pp.tile([P, 2, W], bf16, tag="s2", bufs=1)
                nc.vector.tensor_tensor(s2[:], s5[:, 0:2, :], s5[:, 2:4, :], A.add)
                Dn = pp.tile([P, W], bf16, tag="Dn")
                nc.vector.tensor_tensor(Dn[:], s2[:, 0, :], s2[:, 1, :], A.add)
                nc.vector.tensor_tensor(Dn[:], Dn[:], s5[:, 4, :], A.add)
                nc.vector.tensor_tensor(Dn[:], Dn[:], E[:, 10, :], A.add)
                lnD = pp.tile([P, W], bf16, tag="lnD")
                nc.scalar.activation(lnD[:], Dn[:], AF.Ln)
                r = pp.tile([P, W], bf16, tag="r")
                nc.scalar.activation(r[:], lnD[:], AF.Exp, scale=-1.0)
                # Pr = E * r (in place)
                nc.vector.tensor_tensor(
                    E[:], E[:], r[:].unsqueeze(1).broadcast_to([P, C, W]), A.mult
                )
                Pr = E

                OH = opool.tile([P, C, W], bf16, tag="OH")
                for c in range(C):
                    nc.vector.tensor_scalar(
                        OH[:, c, :], Tb[:], float(c), None, A.is_equal
                    )
                Q = qpool.tile([P, C, W], bf16, tag="Q")
                nc.vector.tensor_tensor(Q[:], OH[:], Pr[:], A.mult)

                # per-class sums that don't depend on later products
                for c in range(C):
                    mm(0, c, OH[:, c, :], k, c)
                for c in range(C):
                    mm(1, c, Q[:, c, :], k, c)
                for c in range(C):
                    mm(2, c, Pr[:, c, :], k, c)

                # pt = sum_c Q (exact: one nonzero per pixel)
                p5 = pp.tile([P, 5, W], bf16, tag="p5", bufs=1)
                nc.vector.tensor_tensor(p5[:], Q[:, 0:5, :], Q[:, 5:10, :], A.add)
                p2 = pp.tile([P, 2, W], bf16, tag="p2", bufs=1)
                nc.vector.tensor_tensor(p2[:], p5[:, 0:2, :], p5[:, 2:4, :], A.add)
                pt = pp.tile([P, W], bf16, tag="pt")
                nc.vector.tensor_tensor(pt[:], p2[:, 0, :], p2[:, 1, :], A.add)
                nc.vector.tensor_tensor(pt[:], pt[:], p5[:, 4, :], A.add)
                nc.vector.tensor_tensor(pt[:], pt[:], Q[:, 10, :], A.add)
                nc.vector.tensor_scalar_max(pt[:], pt[:], 1e-7)
                Lp = pp.tile([P, W], bf16, tag="Lp")
                nc.scalar.activation(Lp[:], pt[:], AF.Ln)
                u2 = pp.tile([P, W], bf16, tag="u2")
                nc.scalar.activation(u2[:], pt[:], AF.Square, bias=1.0, scale=-1.0)
                fpl = pp.tile([P, W], bf16, tag="fpl", bufs=1)
                nc.gpsimd.tensor_tensor(fpl[:], u2[:], Lp[:], A.mult)
                nc.vector.reduce_sum(stats[:, k : k + 1], fpl[:], axis=AX)

                # ---- argmax via bf16 max tree over Pr (ties: multi-hot,
                # verified 6e-4 end-to-end error in numpy) ----
                m5 = pp.tile([P, 5, W], bf16, tag="m5", bufs=1)
                nc.vector.tensor_tensor(m5[:], Pr[:, 0:5, :], Pr[:, 5:10, :], A.max)
                nc.vector.tensor_tensor(
                    m5[:, 0:2, :], m5[:, 0:2, :], m5[:, 2:4, :], A.max
                )
                Em = pp.tile([P, W], bf16, tag="Em", bufs=1)
                nc.vector.tensor_tensor(Em[:], m5[:, 0, :], m5[:, 1, :], A.max)
                nc.vector.tensor_tensor(Em[:], Em[:], m5[:, 4, :], A.max)
                nc.vector.tensor_tensor(Em[:], Em[:], Pr[:, 10, :], A.max)
                PRED = ppool.tile([P, C, W], bf16, tag="PRED")
                nc.vector.tensor_tensor(
                    PRED[:], Pr[:], Em[:].unsqueeze(1).broadcast_to([P, C, W]),
                    A.is_equal,
                )
                # PWQ = PRED * 2^c as exact i16 words
                PWQ = wpool.tile([P, C, W], i16, tag="PWQ")
                for c in range(C):
                    nc.scalar.activation(
                        PWQ[:, c, :], PRED[:, c, :], AF.Copy, scale=float(1 << c)
                    )

                # ---- word planes (i16) ----
                mwUD = pp.tile([P, 2, W], i16, tag="mwUD", bufs=1)
                nc.vector.tensor_tensor(
                    mwUD[:], ones_i[:].unsqueeze(1).broadcast_to([P, 2, W]),
                    TUD[:], A.logical_shift_left,
                )
                roUD = pp.tile([P, 2, W], i16, tag="roUD", bufs=1)
                nc.vector.memset(roUD[:, :, 0:1], 0)
                nc.vector.tensor_copy(roUD[:, :, 1:W], mwUD[:, :, 0 : W - 1])
                nc.vector.tensor_tensor(roUD[:], roUD[:], mwUD[:], A.bitwise_or)
                nc.vector.tensor_tensor(
                    roUD[:, :, 0 : W - 1], roUD[:, :, 0 : W - 1],
                    mwUD[:, :, 1:W], A.bitwise_or,
                )
                arUD = pp.tile([P, 2, W], i16, tag="arUD", bufs=1)
                nc.vector.memset(arUD[:, :, 0:1], 0)
                nc.vector.memset(arUD[:, :, W - 1 : W], 0)
                nc.vector.tensor_tensor(
                    arUD[:, :, 1 : W - 1], mwUD[:, :, 1 : W - 1],
                    mwUD[:, :, 0 : W - 2], A.bitwise_and,
                )
                nc.vector.tensor_tensor(
                    arUD[:, :, 1 : W - 1], arUD[:, :, 1 : W - 1],
                    mwUD[:, :, 2:W], A.bitwise_and,
                )
                mwC = pp.tile([P, W], i16, tag="mwC", bufs=1)
                nc.vector.tensor_tensor(mwC[:], ones_i[:], T[:], A.logical_shift_left)
                rwC = pp.tile([P, W], i16, tag="rwC", bufs=1)
                nc.vector.memset(rwC[:, 0:1], 0)
                nc.vector.tensor_copy(rwC[:, 1:W], mwC[:, 0 : W - 1])
                nc.vector.tensor_tensor(
                    rwC[:, 0 : W - 1], rwC[:, 0 : W - 1], mwC[:, 1:W], A.bitwise_or
                )
                roC = pp.tile([P, W], i16, tag="roC", bufs=1)
                nc.vector.tensor_tensor(roC[:], rwC[:], mwC[:], A.bitwise_or)
                arC = pp.tile([P, W], i16, tag="arC", bufs=1)
                nc.vector.memset(arC[:, 0:1], 0)
                nc.vector.memset(arC[:, W - 1 : W], 0)
                nc.vector.tensor_tensor(
                    arC[:, 1 : W - 1], mwC[:, 1 : W - 1], mwC[:, 0 : W - 2],
                    A.bitwise_and,
                )
                nc.vector.tensor_tensor(
                    arC[:, 1 : W - 1], arC[:, 1 : W - 1], mwC[:, 2:W], A.bitwise_and
                )
                or8 = pp.tile([P, W], i16, tag="or8", bufs=1)
                nc.vector.tensor_tensor(or8[:], roUD[:, 0, :], roUD[:, 1, :], A.bitwise_or)
                nc.vector.tensor_tensor(or8[:], or8[:], roC[:], A.bitwise_or)
                an9 = pp.tile([P, W], i16, tag="an9", bufs=1)
                nc.vector.tensor_tensor(an9[:], arUD[:, 0, :], arUD[:, 1, :], A.bitwise_and)
                nc.vector.tensor_tensor(an9[:], an9[:], arC[:], A.bitwise_and)
                or4 = pp.tile([P, W], i16, tag="or4", bufs=1)
                nc.vector.tensor_tensor(or4[:], mwUD[:, 0, :], mwUD[:, 1, :], A.bitwise_or)
                nc.vector.tensor_tensor(or4[:], or4[:], rwC[:], A.bitwise_or)

                # pmi = 1 << pred (i16 tree over PWQ)
                w5 = pp.tile([P, 5, W], i16, tag="w5", bufs=1)
                nc.vector.tensor_tensor(w5[:], PWQ[:, 0:5, :], PWQ[:, 5:10, :], A.add)
                w2 = pp.tile([P, 2, W], i16, tag="w2", bufs=1)
                nc.vector.tensor_tensor(w2[:], w5[:, 0:2, :], w5[:, 2:4, :], A.add)
                pmi = pp.tile([P, W], i16, tag="pmi", bufs=1)
                nc.vector.tensor_tensor(pmi[:], w2[:, 0, :], w2[:, 1, :], A.add)
                nc.vector.tensor_tensor(pmi[:], pmi[:], w5[:, 4, :], A.add)
                nc.vector.tensor_tensor(pmi[:], pmi[:], PWQ[:, 10, :], A.add)

                # npe = [pred != t], gAp = npe * boundary_t
                ti = pp.tile([P, W], i16, tag="ti", bufs=1)
                nc.vector.tensor_tensor(ti[:], mwC[:], pmi[:], A.bitwise_and)
                npe = pp.tile([P, W], bf16, tag="npe", bufs=1)
                nc.vector.tensor_scalar(npe[:], ti[:], 0, None, A.is_equal)
                b9tp = pp.tile([P, W], bf16, tag="b9tp", bufs=1)
                nc.vector.tensor_tensor(b9tp[:], an9[:], mwC[:], A.not_equal)
                gAp = pp.tile([P, W], bf16, tag="gAp", bufs=1)
                nc.vector.tensor_tensor(gAp[:], npe[:], b9tp[:], A.mult)

                # g23 = (ES + (E1-ES)*O4) * B0 * npe
                o4a = pp.tile([P, W], i16, tag="o4a", bufs=1)
                nc.vector.tensor_tensor(o4a[:], or4[:], pmi[:], A.bitwise_and)
                g23 = pp.tile([P, W], bf16, tag="g23", bufs=1)
                nc.vector.tensor_scalar(g23[:], o4a[:], 0, None, A.is_gt)
                nc.vector.tensor_scalar(g23[:], g23[:], E1 - ES, ES, A.mult, A.add)
                b0a = pp.tile([P, W], i16, tag="b0a", bufs=1)
                nc.vector.tensor_tensor(b0a[:], or8[:], pmi[:], A.bitwise_and)
                b0v = pp.tile([P, W], bf16, tag="b0v", bufs=1)
                nc.vector.tensor_scalar(b0v[:], b0a[:], 0, None, A.is_gt)
                nc.vector.tensor_tensor(g23[:], g23[:], b0v[:], A.mult)
                nc.vector.tensor_tensor(g23[:], g23[:], npe[:], A.mult)

                # GR = OH * gAp (in place), NRt = PRED * g23 (in place)
                nc.gpsimd.tensor_tensor(
                    OH[:], OH[:], gAp[:].unsqueeze(1).broadcast_to([P, C, W]), A.mult
                )
                nc.gpsimd.tensor_tensor(
                    PRED[:], PRED[:], g23[:].unsqueeze(1).broadcast_to([P, C, W]),
                    A.mult,
                )
                for c in range(C):
                    mm(3, c, OH[:, c, :], k, c)
                for c in range(C):
                    mm(4, c, PRED[:, c, :], k, c)

                # ne: BW = ~an9 & or8, bit-sliced per class
                nn = pp.tile([P, W], i16, tag="nn", bufs=1)
                nc.vector.tensor_scalar(nn[:], an9[:], -1, None, A.bitwise_xor)
                BW = pp.tile([P, W], i16, tag="BW", bufs=1)
                nc.vector.tensor_tensor(BW[:], nn[:], or8[:], A.bitwise_and)
                BWbi = wpool.tile([P, C, W], i16, tag="PWQ", name="BWbi")
                for c in range(C):
                    nc.vector.tensor_scalar(
                        BWbi[:, c, :], BW[:], c, 1,
                        A.logical_shift_right, A.bitwise_and,
                    )
                BWb = bbuf.tile([P, C, W], bf16, tag="BWb")
                nc.scalar.copy(BWb[:], BWbi[:])
                for c in range(C):
                    mm(5, c, BWb[:, c, :], k, c)

            # evacuate PSUM accumulators (chunked to keep SBUF small)
            for q in range(NQ):
                ev = pp.tile([C, W], f32, tag="ev", name="ev")
                nc.scalar.copy(ev[:], ps[q][:])
                nc.sync.dma_start(pso[:, q * W : (q + 1) * W], ev[:])
            nc.sync.dma_start(sto[:], stats[:])

    nc.compile()
    return nc


def _host_combine(results):
    soh = np.zeros((B, C)); inter = np.zeros((B, C)); sumP = np.zeros((B, C))
    gA = np.zeros((B, C)); NR = np.zeros((B, C)); ne = np.zeros((B, C))
    fsum = np.zeros(B)
    for b in range(B):
        psums = results[b]["psums"].astype(np.float64)  # [C, 6*W]
        soh[b] = psums[:, 0 * W : 1 * W].sum(axis=1)
        inter[b] = psums[:, 1 * W : 2 * W].sum(axis=1)
        sumP[b] = psums[:, 2 * W : 3 * W].sum(axis=1)
        gA[b] = psums[:, 3 * W : 4 * W].sum(axis=1)
        NR[b] = psums[:, 4 * W : 5 * W].sum(axis=1)
        ne[b] = psums[:, 5 * W : 6 * W].sum(axis=1)
        fsum[b] = results[b]["stats"].astype(np.float64).sum()

    cls = np.arange(C)
    dice = (2.0 * inter + EPS) / (sumP + soh + EPS)
    cls_valid = (soh.sum(axis=0) > 0) & (cls != 0)
    nvalid = int(cls_valid.sum())
    dice_score = (dice.mean(axis=0) * cls_valid).sum() / max(nvalid, 1)
    dice_loss = (1.0 - dice_score) if nvalid > 0 else 0.0

    focal_loss = -FOCAL_ALPHA * fsum.sum() / (B * H * W)

    werr = gA + NR
    class_loss = werr / np.maximum(ne, 1.0)
    valid_bc = (soh > 0) & (cls[None, :] != 0)
    nvalid_b = valid_bc.sum(axis=1)
    sample = (class_loss * valid_bc).sum(axis=1) / np.maximum(nvalid_b, 1)
    edge_loss = float(np.where(nvalid_b > 0, sample, 0.0).mean())

    total = dice_loss + focal_loss + edge_loss
    return (
        np.float32(total),
        np.float32(dice_loss),
        np.float32(focal_loss),
        np.float32(edge_loss),
    )


def kernel(inputs, targets):
    import ml_dtypes
    from concourse.bass_utils import run_bass_kernel_spmd

    inputs = np.ascontiguousarray(np.asarray(inputs, dtype=np.float32))
    tgt = np.asarray(targets)
    t16 = np.ascontiguousarray(tgt.astype(np.int16))
    tbf = np.ascontiguousarray(tgt.astype(ml_dtypes.bfloat16))

    nc = _build()
    in_maps = [
        {"inputs": inputs[b], "t16": t16[b], "tbf": tbf[b]} for b in range(B)
    ]
    res = run_bass_kernel_spmd(nc, in_maps, core_ids=list(range(B)))
    return _host_combine(res.results)


# revision 12
# speedup vs baseline: 1.6611x; 1.0125x over previous
"""Compound loss (dice + focal + edge) kernel for Trainium2, 8-core data-parallel.

Shapes hardcoded: inputs [8, 11, 512, 512] f32, targets [8, 512, 512] int.
Each NeuronCore processes one batch sample; per-class reductions run on the
TensorEngine (one-hot-column stationary matmuls accumulating into PSUM
[11, 512] banks); the tiny cross-batch combination happens on host.

v2 design (vs v1 at 464 us: Vector 89% / Scalar 72% busy):
- all per-class column sums -> TensorE matmul (lhsT = [128,11] one-hot col c,
  rhs = quantity plane [128,512], PSUM accumulates across row-tiles).
- strided C-reductions -> contiguous pairwise trees on bf16 (2x DVE rate).
- argmax (max tree + is_equal) on GpSimd in f32 (exact, engine otherwise idle).
- pred bit-word via ScalarE per-class scale 2^c into i16 + Vector tree (exact).
- 3x3 word-plane convs: horizontal ops on i16 words; vertical neighbors by
  re-loading row-shifted target slices from DRAM (no round-trip of planes).
- host passes targets pre-cast as i16 and bf16 to skip on-device casts.
"""

import sys

sys.path.insert(0, "/opt/trn_rl_repo")

import functools
import numpy as np

B, C, H, W = 8, 11, 512, 512
P = 128
NT = H // P
EPS = 1e-6
FOCAL_ALPHA = 0.25
E1 = float(np.exp(-1.0))
ES = float(np.exp(-np.sqrt(2.0)))

NQ = 6  # soh, inter, sumP, gA, NR, ne


@functools.cache
def _build():
    import concourse.bacc as bacc
    from concourse import mybir, tile

    f32 = mybir.dt.float32
    bf16 = mybir.dt.bfloat16
    i16 = mybir.dt.int16
    A = mybir.AluOpType
    AF = mybir.ActivationFunctionType
    AX = mybir.AxisListType.X

    nc = bacc.Bacc(None, target_bir_lowering=False)
    xin = nc.dram_tensor("inputs", [C, H, W], f32, kind="ExternalInput")
    t16 = nc.dram_tensor("t16", [H, W], i16, kind="ExternalInput")
    tbf = nc.dram_tensor("tbf", [H, W], bf16, kind="ExternalInput")
    pso = nc.dram_tensor("psums", [C, NQ * W], f32, kind="ExternalOutput")
    sto = nc.dram_tensor("stats", [P, NT], f32, kind="ExternalOutput")

    with tile.TileContext(nc) as tc:
        with (
            tc.tile_pool(name="const", bufs=1) as cpool,
            tc.tile_pool(name="xbuf", bufs=2) as xpool,
            tc.tile_pool(name="ebuf", bufs=2) as epool,
            tc.tile_pool(name="obuf", bufs=1) as opool,
            tc.tile_pool(name="qbuf", bufs=1) as qpool,
            tc.tile_pool(name="pbuf", bufs=1) as ppool,
            tc.tile_pool(name="wbuf", bufs=1) as wpool,
            tc.tile_pool(name="bbuf", bufs=1) as bbuf,
            tc.tile_pool(name="pl", bufs=2) as pp,
            tc.tile_pool(name="tb", bufs=1) as tpool,
            tc.psum_pool(name="acc", bufs=1) as psp,
        ):
            ones_i = cpool.tile([P, W], i16)
            nc.vector.memset(ones_i[:], 1)
            # IDE[:, c, :] = one-hot row pattern: column c ones (stationary)
            IDE = cpool.tile([P, C, C], bf16)
            nc.vector.memset(IDE[:], 0.0)
            for c in range(C):
                nc.vector.memset(IDE[:, c, c : c + 1], 1.0)
            stats = cpool.tile([P, NT], f32)

            ps = [
                psp.tile([C, W], f32, tag=f"ps{q}", name=f"ps{q}")
                for q in range(NQ)
            ]

            def mm(q, lhs_c, rhs, k, c):
                nc.tensor.matmul(
                    ps[q][:],
                    IDE[:, lhs_c, :],
                    rhs,
                    start=(k == 0 and c == 0),
                    stop=(k == NT - 1 and c == C - 1),
                )

            for k in range(NT):
                h0 = k * P

                Xt = xpool.tile([P, C, W], f32, tag="X")
                nc.sync.dma_start(
                    Xt[:], xin[:, h0 : h0 + P, :].rearrange("c h w -> h c w")
                )
                T = tpool.tile([P, W], i16, tag="T")
                nc.sync.dma_start(T[:], t16[h0 : h0 + P, :])
                Tb = tpool.tile([P, W], bf16, tag="Tb")
                nc.sync.dma_start(Tb[:], tbf[h0 : h0 + P, :])
                TUD = tpool.tile([P, 2, W], i16, tag="TUD")
                if k == 0:
                    nc.vector.memset(TUD[0:1, 0, :], 0)
                    nc.sync.dma_start(TUD[1:P, 0, :], t16[0 : P - 1, :])
                else:
                    nc.sync.dma_start(TUD[:, 0, :], t16[h0 - 1 : h0 + P - 1, :])
                if k == NT - 1:
                    nc.vector.memset(TUD[:, 1, :], 0)
                    nc.sync.dma_start(TUD[0 : P - 1, 1, :], t16[h0 + 1 : H, :])
                else:
                    nc.sync.dma_start(TUD[:, 1, :], t16[h0 + 1 : h0 + P + 1, :])

                # ---- softmax pieces ----
                E = epool.tile([P, C, W], bf16, tag="E")
                nc.scalar.activation(E[:], Xt[:], AF.Exp)
                s5 = pp.tile([P, 5, W], bf16, tag="s5", bufs=1)
                nc.vector.tensor_tensor(s5[:], E[:, 0:5, :], E[:, 5:10, :], A.add)
                s2 = pp.tile([P, 2, W], bf16, tag="s2", bufs=1)
                nc.vector.tensor_tensor(s2[:], s5[:, 0:2, :], s5[:, 2:4, :], A.add)
                Dn = pp.tile([P, W], bf16, tag="Dn")
                nc.vector.tensor_tensor(Dn[:], s2[:, 0, :], s2[:, 1, :], A.add)
                nc.vector.tensor_tensor(Dn[:], Dn[:], s5[:, 4, :], A.add)
                nc.vector.tensor_tensor(Dn[:], Dn[:], E[:, 10, :], A.add)
                lnD = pp.tile([P, W], bf16, tag="lnD")
                nc.scalar.activation(lnD[:], Dn[:], AF.Ln)
                r = pp.tile([P, W], bf16, tag="r")
                nc.scalar.activation(r[:], lnD[:], AF.Exp, scale=-1.0)
                # Pr = E * r (in place)
                nc.vector.tensor_tensor(
                    E[:], E[:], r[:].unsqueeze(1).broadcast_to([P, C, W]), A.mult
                )
                Pr = E

                OH = opool.tile([P, C, W], bf16, tag="OH")
                for c in range(C):
                    nc.vector.tensor_scalar(
                        OH[:, c, :], Tb[:], float(c), None, A.is_equal
                    )
                Q = qpool.tile([P, C, W], bf16, tag="Q")
                nc.vector.tensor_tensor(Q[:], OH[:], Pr[:], A.mult)

                # per-class sums that don't depend on later products
                for c in range(C):
                    mm(0, c, OH[:, c, :], k, c)
                for c in range(C):
                    mm(1, c, Q[:, c, :], k, c)
                for c in range(C):
                    mm(2, c, Pr[:, c, :], k, c)

                # pt = sum_c Q (exact: one nonzero per pixel)
                p5 = pp.tile([P, 5, W], bf16, tag="p5", bufs=1)
                nc.vector.tensor_tensor(p5[:], Q[:, 0:5, :], Q[:, 5:10, :], A.add)
                p2 = pp.tile([P, 2, W], bf16, tag="p2", bufs=1)
                nc.vector.tensor_tensor(p2[:], p5[:, 0:2, :], p5[:, 2:4, :], A.add)
                pt = pp.tile([P, W], bf16, tag="pt")
                nc.vector.tensor_tensor(pt[:], p2[:, 0, :], p2[:, 1, :], A.add)
                nc.vector.tensor_tensor(pt[:], pt[:], p5[:, 4, :], A.add)
                nc.vector.tensor_tensor(pt[:], pt[:], Q[:, 10, :], A.add)
                nc.vector.tensor_scalar_max(pt[:], pt[:], 1e-7)
                Lp = pp.tile([P, W], bf16, tag="Lp")
                nc.scalar.activation(Lp[:], pt[:], AF.Ln)
                u2 = pp.tile([P, W], bf16, tag="u2")
                nc.scalar.activation(u2[:], pt[:], AF.Square, bias=1.0, scale=-1.0)
                fpl = pp.tile([P, W], bf16, tag="fpl", bufs=1)
                nc.gpsimd.tensor_tensor(fpl[:], u2[:], Lp[:], A.mult)
                nc.vector.reduce_sum(stats[:, k : k + 1], fpl[:], axis=AX)

                # ---- argmax via bf16 max tree over Pr (ties: multi-hot,
                # verified 6e-4 end-to-end error in numpy) ----
                m5 = pp.tile([P, 5, W], bf16, tag="m5", bufs=1)
                nc.vector.tensor_tensor(m5[:], Pr[:, 0:5, :], Pr[:, 5:10, :], A.max)
                nc.vector.tensor_tensor(
                    m5[:, 0:2, :], m5[:, 0:2, :], m5[:, 2:4, :], A.max
                )
                Em = pp.tile([P, W], bf16, tag="Em", bufs=1)
                nc.vector.tensor_tensor(Em[:], m5[:, 0, :], m5[:, 1, :], A.max)
                nc.vector.tensor_tensor(Em[:], Em[:], m5[:, 4, :], A.max)
                nc.vector.tensor_tensor(Em[:], Em[:], Pr[:, 10, :], A.max)
                PRED = ppool.tile([P, C, W], bf16, tag="PRED")
                nc.vector.tensor_tensor(
                    PRED[:], Pr[:], Em[:].unsqueeze(1).broadcast_to([P, C, W]),
                    A.is_equal,
                )
                # PWQ = PRED * 2^c as exact i16 words
                PWQ = wpool.tile([P, C, W], i16, tag="PWQ")
                for c in range(C):
                    nc.scalar.activation(
                        PWQ[:, c, :], PRED[:, c, :], AF.Copy, scale=float(1 << c)
                    )

                # ---- word planes (i16) ----
                mwUD = pp.tile([P, 2, W], i16, tag="mwUD", bufs=1)
                nc.vector.tensor_tensor(
                    mwUD[:], ones_i[:].unsqueeze(1).broadcast_to([P, 2, W]),
                    TUD[:], A.logical_shift_left,
                )
                # separable 3x3: vertical OR/AND first, then horizontal
                mwC = pp.tile([P, W], i16, tag="mwC", bufs=1)
                nc.vector.tensor_tensor(mwC[:], ones_i[:], T[:], A.logical_shift_left)
                vo2 = pp.tile([P, W], i16, tag="vo2", bufs=1)
                nc.vector.tensor_tensor(vo2[:], mwUD[:, 0, :], mwUD[:, 1, :], A.bitwise_or)
                vo3 = pp.tile([P, W], i16, tag="vo3", bufs=1)
                nc.vector.tensor_tensor(vo3[:], vo2[:], mwC[:], A.bitwise_or)
                va = pp.tile([P, W], i16, tag="va", bufs=1)
                nc.vector.tensor_tensor(va[:], mwUD[:, 0, :], mwUD[:, 1, :], A.bitwise_and)
                nc.vector.tensor_tensor(va[:], va[:], mwC[:], A.bitwise_and)
                or8 = pp.tile([P, W], i16, tag="or8", bufs=1)
                nc.vector.tensor_tensor(or8[:], vo3[:], vo3[:], A.bitwise_or)
                nc.vector.tensor_tensor(
                    or8[:, 1:W], or8[:, 1:W], vo3[:, 0 : W - 1], A.bitwise_or
                )
                nc.vector.tensor_tensor(
                    or8[:, 0 : W - 1], or8[:, 0 : W - 1], vo3[:, 1:W], A.bitwise_or
                )
                an9 = pp.tile([P, W], i16, tag="an9", bufs=1)
                nc.vector.memset(an9[:, 0:1], 0)
                nc.vector.memset(an9[:, W - 1 : W], 0)
                nc.vector.tensor_tensor(
                    an9[:, 1 : W - 1], va[:, 1 : W - 1], va[:, 0 : W - 2],
                    A.bitwise_and,
                )
                nc.vector.tensor_tensor(
                    an9[:, 1 : W - 1], an9[:, 1 : W - 1], va[:, 2:W], A.bitwise_and
                )
                or4 = pp.tile([P, W], i16, tag="or4", bufs=1)
                nc.vector.tensor_tensor(or4[:], vo2[:], vo2[:], A.bitwise_or)
                nc.vector.tensor_tensor(
                    or4[:, 1:W], or4[:, 1:W], mwC[:, 0 : W - 1], A.bitwise_or
                )
                nc.vector.tensor_tensor(
                    or4[:, 0 : W - 1], or4[:, 0 : W - 1], mwC[:, 1:W], A.bitwise_or
                )

                # pmi = 1 << pred (i16 tree over PWQ)
                w5 = pp.tile([P, 5, W], i16, tag="w5", bufs=1)
                nc.vector.tensor_tensor(w5[:], PWQ[:, 0:5, :], PWQ[:, 5:10, :], A.add)
                w2 = pp.tile([P, 2, W], i16, tag="w2", bufs=1)
                nc.vector.tensor_tensor(w2[:], w5[:, 0:2, :], w5[:, 2:4, :], A.add)
                pmi = pp.tile([P, W], i16, tag="pmi", bufs=1)
                nc.vector.tensor_tensor(pmi[:], w2[:, 0, :], w2[:, 1, :], A.add)
                nc.vector.tensor_tensor(pmi[:], pmi[:], w5[:, 4, :], A.add)
                nc.vector.tensor_tensor(pmi[:], pmi[:], PWQ[:, 10, :], A.add)

                # npe = [pred != t], gAp = npe * boundary_t
                ti = pp.tile([P, W], i16, tag="ti", bufs=1)
                nc.vector.tensor_tensor(ti[:], mwC[:], pmi[:], A.bitwise_and)
                npe = pp.tile([P, W], bf16, tag="npe", bufs=1)
                nc.vector.tensor_scalar(npe[:], ti[:], 0, None, A.is_equal)
                b9tp = pp.tile([P, W], bf16, tag="b9tp", bufs=1)
                nc.vector.tensor_tensor(b9tp[:], an9[:], mwC[:], A.not_equal)
                gAp = pp.tile([P, W], bf16, tag="gAp", bufs=1)
                nc.vector.tensor_tensor(gAp[:], npe[:], b9tp[:], A.mult)

                # g23 = (ES + (E1-ES)*O4) * B0 * npe
                o4a = pp.tile([P, W], i16, tag="o4a", bufs=1)
                nc.vector.tensor_tensor(o4a[:], or4[:], pmi[:], A.bitwise_and)
                g23 = pp.tile([P, W], bf16, tag="g23", bufs=1)
                nc.vector.tensor_scalar(g23[:], o4a[:], 0, None, A.is_gt)
                nc.vector.tensor_scalar(g23[:], g23[:], E1 - ES, ES, A.mult, A.add)
                b0a = pp.tile([P, W], i16, tag="b0a", bufs=1)
                nc.vector.tensor_tensor(b0a[:], or8[:], pmi[:], A.bitwise_and)
                b0v = pp.tile([P, W], bf16, tag="b0v", bufs=1)
                nc.vector.tensor_scalar(b0v[:], b0a[:], 0, None, A.is_gt)
                nc.vector.tensor_tensor(g23[:], g23[:], b0v[:], A.mult)
                nc.vector.tensor_tensor(g23[:], g23[:], npe[:], A.mult)

                # GR = OH * gAp (in place), NRt = PRED * g23 (in place)
                nc.gpsimd.tensor_tensor(
                    OH[:], OH[:], gAp[:].unsqueeze(1).broadcast_to([P, C, W]), A.mult
                )
                nc.gpsimd.tensor_tensor(
                    PRED[:], PRED[:], g23[:].unsqueeze(1).broadcast_to([P, C, W]),
                    A.mult,
                )
                for c in range(C):
                    mm(3, c, OH[:, c, :], k, c)
                for c in range(C):
                    mm(4, c, PRED[:, c, :], k, c)

                # ne: BW = ~an9 & or8, bit-sliced per class
                nn = pp.tile([P, W], i16, tag="nn", bufs=1)
                nc.vector.tensor_scalar(nn[:], an9[:], -1, None, A.bitwise_xor)
                BW = pp.tile([P, W], i16, tag="BW", bufs=1)
                nc.vector.tensor_tensor(BW[:], nn[:], or8[:], A.bitwise_and)
                BWbi = wpool.tile([P, C, W], i16, tag="PWQ", name="BWbi")
                for c in range(C):
                    nc.vector.tensor_scalar(
                        BWbi[:, c, :], BW[:], c, 1,
                        A.logical_shift_right, A.bitwise_and,
                    )
                BWb = bbuf.tile([P, C, W], bf16, tag="BWb")
                nc.scalar.copy(BWb[:], BWbi[:])
                for c in range(C):
                    mm(5, c, BWb[:, c, :], k, c)

            # evacuate PSUM accumulators (chunked to keep SBUF small)
            for q in range(NQ):
                ev = pp.tile([C, W], f32, tag="ev", name="ev")
                nc.scalar.copy(ev[:], ps[q][:])
                nc.sync.dma_start(pso[:, q * W : (q + 1) * W], ev[:])
            nc.sync.dma_start(sto[:], stats[:])

    nc.compile()
    return nc


def _host_combine(results):
    soh = np.zeros((B, C)); inter = np.zeros((B, C)); sumP = np.zeros((B, C))
    gA = np.zeros((B, C)); NR = np.zeros((B, C)); ne = np.zeros((B, C))
    fsum = np.zeros(B)
    for b in range(B):
        psums = results[b]["psums"].astype(np.float64)  # [C, 6*W]
        soh[b] = psums[:, 0 * W : 1 * W].sum(axis=1)
        inter[b] = psums[:, 1 * W : 2 * W].sum(axis=1)
        sumP[b] = psums[:, 2 * W : 3 * W].sum(axis=1)
        gA[b] = psums[:, 3 * W : 4 * W].sum(axis=1)
        NR[b] = psums[:, 4 * W : 5 * W].sum(axis=1)
        ne[b] = psums[:, 5 * W : 6 * W].sum(axis=1)
        fsum[b] = results[b]["stats"].astype(np.float64).sum()

    cls = np.arange(C)
    dice = (2.0 * inter + EPS) / (sumP + soh + EPS)
    cls_valid = (soh.sum(axis=0) > 0) & (cls != 0)
    nvalid = int(cls_valid.sum())
    dice_score = (dice.mean(axis=0) * cls_valid).sum() / max(nvalid, 1)
    dice_loss = (1.0 - dice_score) if nvalid > 0 else 0.0

    focal_loss = -FOCAL_ALPHA * fsum.sum() / (B * H * W)

    werr = gA + NR
    class_loss = werr / np.maximum(ne, 1.0)
    valid_bc = (soh > 0) & (cls[None, :] != 0)
    nvalid_b = valid_bc.sum(axis=1)
    sample = (class_loss * valid_bc).sum(axis=1) / np.maximum(nvalid_b, 1)
    edge_loss = float(np.where(nvalid_b > 0, sample, 0.0).mean())

    total = dice_loss + focal_loss + edge_loss
    return (
        np.float32(total),
        np.float32(dice_loss),
        np.float32(focal_loss),
        np.float32(edge_loss),
    )


def kernel(inputs, targets):
    import ml_dtypes
    from concourse.bass_utils import run_bass_kernel_spmd

    inputs = np.ascontiguousarray(np.asarray(inputs, dtype=np.float32))
    tgt = np.asarray(targets)
    t16 = np.ascontiguousarray(tgt.astype(np.int16))
    tbf = np.ascontiguousarray(tgt.astype(ml_dtypes.bfloat16))

    nc = _build()
    in_maps = [
        {"inputs": inputs[b], "t16": t16[b], "tbf": tbf[b]} for b in range(B)
    ]
    res = run_bass_kernel_spmd(nc, in_maps, core_ids=list(range(B)))
    return _host_combine(res.results)
